# revision 5
# baseline (speedup 1.0000x reference)
"""Adaptive bilateral filter (nn_AdaptiveFilter) on 8 TRN2 NeuronCores.

Math: out_c(p) = sum_k x_c(p+d_k) * wt_k(p) / sum_k wt_k(p)
with wt_k = softmax_k(w)(p) * exp(-50 * (sum_c |g_c(p+d_k) - g_c(p)|)^2).
Softmax normalization (and its max-subtraction) cancels in num/den, so
wt_k = exp(w0[src(k)]) * exp(-50*s^2) with src = reflect map (7,7)->(4,4).

Sharding: 8 cores = 2 batches x 4 row-bands of 128 rows. Host reflect-pads
to (518,518) and ships each core a (3,134,518) band of x and guidance (halo
included), plus its (128, 512*16) slice of w0. No collectives.
"""
import sys
sys.path.insert(0, "/opt/trn_rl_repo")
import math
import numpy as np

import concourse.bacc as bacc
import concourse.mybir as mybir
import concourse.tile as tile
from concourse.bass_utils import run_bass_kernel_spmd

F32 = mybir.dt.float32
AF = mybir.ActivationFunctionType
OP = mybir.AluOpType

KH = KW = 7
H_BAND = 128
W = 512
WP = 518
SCALE = math.sqrt(50.0)  # Square(sqrt(50)*s) = 50*s^2

_CACHE = {}


def _emit(nc, constp, gxp, workp, finp, psump, g_d, x_d, w_d, id_d, out_d):
    ident = constp.tile([128, 128], F32, tag="ident", name="ident")
    nc.sync.dma_start(ident[:], id_d.ap()[:, :])

    # E[:, t*512:(t+1)*512] = exp(w0 source tap t), t = ti*4+tj
    wraw = constp.tile([H_BAND, W * 16], F32, tag="wraw", name="wraw")
    nc.sync.dma_start(wraw[:], w_d.ap()[:, :])
    E = constp.tile([H_BAND, 16 * W], F32, tag="E", name="E")
    wv = wraw[:].rearrange("p (w s) -> p s w", s=16)
    for t in range(16):
        nc.scalar.activation(E[:, t * W:(t + 1) * W], wv[:, t, :], AF.Exp)

    # center guidance rows (shift i=3), full padded width
    gc = []
    for ch in range(3):
        t = constp.tile([H_BAND, WP], F32, tag=f"gc{ch}", name=f"gc{ch}")
        nc.sync.dma_start(t[:], g_d.ap()[ch, 3:3 + H_BAND, :])
        gc.append(t)

    den_ps = psump.tile([H_BAND, W], F32, tag="dps", name="dps")
    num_ps = [psump.tile([H_BAND, W], F32, tag=f"nps{c}", name=f"nps{c}")
              for c in range(3)]

    for i in range(KH):
        gi, xi = [], []
        for ch in range(3):
            t = gxp.tile([H_BAND, WP], F32, tag=f"g{ch}", name=f"gt{ch}")
            nc.sync.dma_start(t[:], g_d.ap()[ch, i:i + H_BAND, :])
            gi.append(t)
        for ch in range(3):
            t = gxp.tile([H_BAND, WP], F32, tag=f"x{ch}", name=f"xt{ch}")
            nc.sync.dma_start(t[:], x_d.ap()[ch, i:i + H_BAND, :])
            xi.append(t)
        ri = min(i, 6 - i)
        for j in range(KW):
            first = (i == 0 and j == 0)
            last = (i == 6 and j == 6)
            t_src = ri * 4 + min(j, 6 - j)

            a = []
            for ch in range(3):
                u = workp.tile([H_BAND, W], F32, tag=f"u{ch}", name=f"u{ch}")
                nc.vector.tensor_tensor(
                    u[:], gi[ch][:, j:j + W], gc[ch][:, 3:3 + W], OP.subtract)
                av = workp.tile([H_BAND, W], F32, tag=f"a{ch}", name=f"a{ch}")
                nc.vector.scalar_tensor_tensor(
                    av[:], u[:], -1.0, u[:], OP.mult, OP.max)
                a.append(av)
            s01 = workp.tile([H_BAND, W], F32, tag="s01", name="s01")
            nc.vector.tensor_tensor(s01[:], a[0][:], a[1][:], OP.add)
            s = workp.tile([H_BAND, W], F32, tag="s", name="s")
            nc.vector.tensor_tensor(s[:], s01[:], a[2][:], OP.add)

            sq = workp.tile([H_BAND, W], F32, tag="sq", name="sq")
            nc.scalar.activation(sq[:], s[:], AF.Square, scale=SCALE)
            col = workp.tile([H_BAND, W], F32, tag="col", name="col")
            nc.scalar.activation(col[:], sq[:], AF.Exp, scale=-1.0)

            wt = workp.tile([H_BAND, W], F32, tag="wt", name="wt")
            nc.vector.tensor_tensor(
                wt[:], col[:], E[:, t_src * W:(t_src + 1) * W], OP.mult)
            nc.tensor.matmul(den_ps[:], ident[:], wt[:], start=first, stop=last)
            for ch in range(3):
                prod = workp.tile([H_BAND, W], F32, tag=f"pr{ch}", name=f"pr{ch}")
                nc.vector.tensor_tensor(
                    prod[:], xi[ch][:, j:j + W], wt[:], OP.mult)
                nc.tensor.matmul(num_ps[ch][:], ident[:], prod[:],
                                 start=first, stop=last)

    rec = finp.tile([H_BAND, W], F32, tag="rec", name="rec")
    nc.vector.reciprocal(rec[:], den_ps[:])
    for ch in range(3):
        o = finp.tile([H_BAND, W], F32, tag=f"o{ch}", name=f"o{ch}")
        nc.vector.tensor_tensor(o[:], num_ps[ch][:], rec[:], OP.mult)
        nc.sync.dma_start(out_d.ap()[ch, :, :], o[:])


def _build(reps=1):
    nc = bacc.Bacc("TRN2", target_bir_lowering=False, debug=False)
    g_d = nc.dram_tensor("g", [3, 134, WP], F32, kind="ExternalInput")
    x_d = nc.dram_tensor("x", [3, 134, WP], F32, kind="ExternalInput")
    w_d = nc.dram_tensor("w", [H_BAND, W * 16], F32, kind="ExternalInput")
    id_d = nc.dram_tensor("ident", [128, 128], F32, kind="ExternalInput")
    out_d = nc.dram_tensor("out", [3, H_BAND, W], F32, kind="ExternalOutput")

    with tile.TileContext(nc) as tc:
        with (
            tc.tile_pool(name="const", bufs=1) as constp,
            tc.tile_pool(name="gx", bufs=2) as gxp,
            tc.tile_pool(name="work", bufs=2) as workp,
            tc.tile_pool(name="fin", bufs=1) as finp,
            tc.tile_pool(name="psum", bufs=1, space="PSUM") as psump,
        ):
            for _rep in range(reps):
                _emit(nc, constp, gxp, workp, finp, psump,
                      g_d, x_d, w_d, id_d, out_d)

    nc.compile()
    return nc


def _shard_inputs(x, guidance, w0):
    pad = ((0, 0), (0, 0), (3, 3), (3, 3))
    xp = np.pad(x, pad, mode="reflect")
    gp = np.pad(guidance, pad, mode="reflect")
    ident = np.eye(128, dtype=np.float32)

    in_maps = []
    for c in range(8):
        b, band = divmod(c, 4)
        r0 = band * H_BAND
        in_maps.append({
            "g": np.ascontiguousarray(gp[b, :, r0:r0 + H_BAND + 6, :]),
            "x": np.ascontiguousarray(xp[b, :, r0:r0 + H_BAND + 6, :]),
            "w": np.ascontiguousarray(
                w0[b, r0 * W:(r0 + H_BAND) * W].reshape(H_BAND, W * 16)),
            "ident": ident,
        })
    return in_maps


def kernel(x, guidance, w0):
    x = np.asarray(x, dtype=np.float32)
    guidance = np.asarray(guidance, dtype=np.float32)
    w0 = np.asarray(w0, dtype=np.float32)
    B, C, H, Wf = x.shape

    if "nc" not in _CACHE:
        _CACHE["nc"] = _build()
    nc = _CACHE["nc"]

    in_maps = _shard_inputs(x, guidance, w0)
    res = run_bass_kernel_spmd(nc, in_maps, core_ids=list(range(8)))

    out = np.empty((B, C, H, Wf), dtype=np.float32)
    for c in range(8):
        b, band = divmod(c, 4)
        r0 = band * H_BAND
        out[b, :, r0:r0 + H_BAND, :] = res.results[c]["out"]
    return out


# revision 14
# speedup vs baseline: 2.0821x; 2.0821x over previous
"""Adaptive bilateral filter (nn_AdaptiveFilter) on 8 TRN2 NeuronCores.

Math: out_c(p) = sum_k x_c(p+d_k) * wt_k(p) / sum_k wt_k(p)
with wt_k = softmax_k(w)(p) * exp(-50 * (sum_c |g_c(p+d_k) - g_c(p)|)^2).
Softmax normalization (and its max-subtraction) cancels in num/den, so
wt_k = exp(w0[src(k)]) * exp(-50*s^2) with src = reflect map (7,7)->(4,4).

Sharding: 8 cores = 2 batches x 4 row-bands of 128 rows. Host reflect-pads
to (518,518) and ships each core a (3,134,518) band of x and guidance (halo
included), plus its (128, 512*16) slice of w0. No collectives.

Engine split (per tap-row i, j-packed over the 7 column taps):
  DVE:    3 bf16 subtracts (sliding-window vs broadcast center), 3 bitwise-abs,
          per-tap wt = col*E, 2 of 3 x*wt products
  GPSIMD: f32->bf16 conversions, 1 of 3 products
  ACT:    Square + Exp per tap (+ 16 exp(w0) at setup)
  PE:     channel-sum of |d| into PSUM, den/num accumulation over 49 taps
"""
import sys
sys.path.insert(0, "/opt/trn_rl_repo")
import math
import numpy as np

import concourse.bacc as bacc
import concourse.mybir as mybir
import concourse.tile as tile
from concourse.ap import AP
from concourse.bass_utils import run_bass_kernel_spmd

F32 = mybir.dt.float32
BF16 = mybir.dt.bfloat16
U16 = mybir.dt.uint16
AF = mybir.ActivationFunctionType
OP = mybir.AluOpType

KH = KW = 7
H_BAND = 128
W = 512
WP = 518
WJ = KW * W  # 3584
SCALE = math.sqrt(50.0)  # Square(sqrt(50)*s) = 50*s^2

_CACHE = {}


def _win(ap_obj, nwin, wsize):
    """[128, C] SBUF AP -> [128, nwin, wsize] overlapping windows (step 1)."""
    base = ap_obj.ap
    assert base[-1][0] == 1
    return AP(tensor=ap_obj.tensor, offset=ap_obj.offset,
              ap=[list(base[0]), [1, nwin], [1, wsize]])


def _emit(nc, tc, constp, gxp, workp, finp, psump, g_d, x_d, w_d, id_d, out_d):
    ident = constp.tile([128, 128], BF16, tag="ident", name="ident")
    nc.sync.dma_start(ident[:], id_d.ap()[:, :])

    # E[:, t*512:(t+1)*512] = exp(w0 source tap t), t = ti*4+tj  (bf16)
    E = constp.tile([H_BAND, 16 * W], BF16, tag="E", name="E")
    with tc.tile_pool(name="wpool", bufs=1) as wpool:
        HW2 = W // 2
        for h in range(2):
            wraw = wpool.tile([H_BAND, HW2 * 16], F32, tag="wraw", name="wraw")
            nc.sync.dma_start(wraw[:], w_d.ap()[:, h * HW2 * 16:(h + 1) * HW2 * 16])
            wv = wraw[:].rearrange("p (w s) -> p s w", s=16)
            for t in range(16):
                nc.scalar.activation(
                    E[:, t * W + h * HW2:t * W + (h + 1) * HW2], wv[:, t, :], AF.Exp)

    # center guidance (shift i=3, cols 3..514), bf16
    gcb = []
    for ch in range(3):
        tf = constp.tile([H_BAND, W], F32, tag=f"gcf{ch}", name=f"gcf{ch}")
        nc.sync.dma_start(tf[:], g_d.ap()[ch, 3:3 + H_BAND, 3:3 + W])
        tb = constp.tile([H_BAND, W], BF16, tag=f"gc{ch}", name=f"gc{ch}")
        nc.scalar.copy(tb[:], tf[:])
        gcb.append(tb)

    den_ps = psump.tile([H_BAND, W], F32, tag="dps", name="dps", bufs=1)
    num_wide = psump.tile([H_BAND, 3 * W], F32, tag="npsw", name="npsw", bufs=1)
    num_ps = [num_wide[:, c * W:(c + 1) * W] for c in range(3)]

    for i in range(KH):
        gib, xib = [], []
        for ch in range(3):
            tf = gxp.tile([H_BAND, WP], F32, tag=f"gf{ch}", name=f"gf{ch}", bufs=3)
            nc.sync.dma_start(tf[:], g_d.ap()[ch, i:i + H_BAND, :])
            tb = gxp.tile([H_BAND, WP], BF16, tag=f"gb{ch}", name=f"gb{ch}", bufs=3)
            nc.scalar.copy(tb[:], tf[:])
            gib.append(tb)
        for ch in range(3):
            tf = gxp.tile([H_BAND, WP], F32, tag=f"xf{ch}", name=f"xf{ch}", bufs=3)
            nc.sync.dma_start(tf[:], x_d.ap()[ch, i:i + H_BAND, :])
            tb = gxp.tile([H_BAND, WP], BF16, tag=f"xb{ch}", name=f"xb{ch}", bufs=3)
            nc.scalar.copy(tb[:], tf[:])
            xib.append(tb)

        ri = min(i, 6 - i)
        first_i, last_i = (i == 0), (i == 6)

        # u_c = g window - center (bf16, j-packed), then |u_c| via sign-bit AND
        ab = []
        for ch in range(3):
            u = workp.tile([H_BAND, WJ], BF16, tag=f"u{ch}", name=f"u{ch}", bufs=2)
            uv = u[:].rearrange("p (n w) -> p n w", n=KW)
            nc.vector.tensor_tensor(
                uv, _win(gib[ch][:, :], KW, W),
                gcb[ch][:, :].unsqueeze(1).broadcast_to([H_BAND, KW, W]),
                OP.subtract)
            nc.vector.tensor_scalar(u[:].bitcast(U16), u[:].bitcast(U16),
                                    0x7FFF, None, OP.bitwise_and)
            ab.append(u)

        wt_wide = workp.tile([H_BAND, WJ], BF16, tag="wt", name="wt", bufs=2)
        col_wide = workp.tile([H_BAND, WJ], BF16, tag="colw", name="colw", bufs=2)
        for j in range(KW):
            # s = sum_c |u_c| via PE accumulation (PSUM f32)
            s_ps = psump.tile([H_BAND, W], F32, tag="sps", name="sps", bufs=3)
            for ch in range(3):
                nc.tensor.matmul(s_ps[:], ident[:], ab[ch][:, j * W:(j + 1) * W],
                                 start=(ch == 0), stop=(ch == 2))
            # Derivative_Erf(sqrt(50)*s) = 2/sqrt(pi) * exp(-50*s^2); the
            # 2/sqrt(pi) factor cancels between num and den.
            nc.scalar.activation(col_wide[:, j * W:(j + 1) * W], s_ps[:],
                                 AF.Derivative_Erf, scale=SCALE)

        # wt = col * E(src tap): j in 0..3 reads E slots 4ri..4ri+3 (step +W),
        # j in 4..6 reads slots 4ri+2..4ri (step -W)
        ebase = E[:].offset
        up = AP(tensor=E[:].tensor, offset=ebase + (4 * ri) * W,
                ap=[[16 * W, H_BAND], [W, 4], [1, W]])
        dn = AP(tensor=E[:].tensor, offset=ebase + (4 * ri + 2) * W,
                ap=[[16 * W, H_BAND], [-W, 3], [1, W]])
        nc.vector.tensor_tensor(
            wt_wide[:, 0:4 * W].rearrange("p (n w) -> p n w", n=4),
            col_wide[:, 0:4 * W].rearrange("p (n w) -> p n w", n=4), up, OP.mult)
        nc.vector.tensor_tensor(
            wt_wide[:, 4 * W:].rearrange("p (n w) -> p n w", n=3),
            col_wide[:, 4 * W:].rearrange("p (n w) -> p n w", n=3), dn, OP.mult)
        for j in range(KW):
            nc.tensor.matmul(den_ps[:], ident[:], wt_wide[:, j * W:(j + 1) * W],
                             start=(first_i and j == 0), stop=(last_i and j == 6))

        # products and num accumulation (j-packed); ch2 on GPSIMD
        for ch in range(3):
            prod = workp.tile([H_BAND, WJ], BF16, tag=f"pr{ch}", name=f"pr{ch}",
                              bufs=1)
            pv = prod[:].rearrange("p (n w) -> p n w", n=KW)
            eng = nc.vector
            eng.tensor_tensor(
                pv, _win(xib[ch][:, :], KW, W),
                wt_wide[:].rearrange("p (n w) -> p n w", n=KW), OP.mult)
            for j in range(KW):
                nc.tensor.matmul(num_ps[ch], ident[:],
                                 prod[:, j * W:(j + 1) * W],
                                 start=(first_i and j == 0),
                                 stop=(last_i and j == 6))

    rec = finp.tile([H_BAND, W], F32, tag="rec", name="rec")
    nc.vector.reciprocal(rec[:], den_ps[:])
    o = finp.tile([H_BAND, 3 * W], F32, tag="ow", name="ow")
    nc.vector.tensor_tensor(
        o[:].rearrange("p (c w) -> p c w", c=3),
        num_wide[:].rearrange("p (c w) -> p c w", c=3),
        rec[:].unsqueeze(1).broadcast_to([H_BAND, 3, W]), OP.mult)
    od = out_d.ap()
    dst = AP(tensor=od.tensor, offset=od.offset,
             ap=[[W, H_BAND], [H_BAND * W, 3], [1, W]])
    nc.sync.dma_start(dst, o[:].rearrange("p (c w) -> p c w", c=3))


def _build(reps=1, loop_n=None):
    nc = bacc.Bacc("TRN2", target_bir_lowering=False, debug=False)
    g_d = nc.dram_tensor("g", [3, 134, WP], F32, kind="ExternalInput")
    x_d = nc.dram_tensor("x", [3, 134, WP], F32, kind="ExternalInput")
    w_d = nc.dram_tensor("w", [H_BAND, W * 16], F32, kind="ExternalInput")
    id_d = nc.dram_tensor("ident", [128, 128], BF16, kind="ExternalInput")
    out_d = nc.dram_tensor("out", [3, H_BAND, W], F32, kind="ExternalOutput")

    with tile.TileContext(nc) as tc:
        with (
            tc.tile_pool(name="const", bufs=1) as constp,
            tc.tile_pool(name="gx", bufs=2) as gxp,
            tc.tile_pool(name="work", bufs=2) as workp,
            tc.tile_pool(name="fin", bufs=1) as finp,
            tc.tile_pool(name="psum", bufs=1, space="PSUM") as psump,
        ):
            if loop_n is not None:
                with tc.For_i(0, loop_n, 1):
                    _emit(nc, tc, constp, gxp, workp, finp, psump,
                          g_d, x_d, w_d, id_d, out_d)
            else:
                for _rep in range(reps):
                    _emit(nc, tc, constp, gxp, workp, finp, psump,
                          g_d, x_d, w_d, id_d, out_d)

    nc.compile()
    return nc


def _shard_inputs(x, guidance, w0):
    import ml_dtypes
    pad = ((0, 0), (0, 0), (3, 3), (3, 3))
    xp = np.pad(x, pad, mode="reflect")
    gp = np.pad(guidance, pad, mode="reflect")
    ident = np.eye(128, dtype=ml_dtypes.bfloat16)

    in_maps = []
    for c in range(8):
        b, band = divmod(c, 4)
        r0 = band * H_BAND
        in_maps.append({
            "g": np.ascontiguousarray(gp[b, :, r0:r0 + H_BAND + 6, :]),
            "x": np.ascontiguousarray(xp[b, :, r0:r0 + H_BAND + 6, :]),
            "w": np.ascontiguousarray(
                w0[b, r0 * W:(r0 + H_BAND) * W].reshape(H_BAND, W * 16)),
            "ident": ident,
        })
    return in_maps


def kernel(x, guidance, w0):
    x = np.asarray(x, dtype=np.float32)
    guidance = np.asarray(guidance, dtype=np.float32)
    w0 = np.asarray(w0, dtype=np.float32)
    B, C, H, Wf = x.shape

    if "nc" not in _CACHE:
        _CACHE["nc"] = _build()
    nc = _CACHE["nc"]

    in_maps = _shard_inputs(x, guidance, w0)
    res = run_bass_kernel_spmd(nc, in_maps, core_ids=list(range(8)))

    out = np.empty((B, C, H, Wf), dtype=np.float32)
    for c in range(8):
        b, band = divmod(c, 4)
        r0 = band * H_BAND
        out[b, :, r0:r0 + H_BAND, :] = res.results[c]["out"]
    return out
